# revision 29
# baseline (speedup 1.0000x reference)
"""Trainium2 Bass kernel for ConvChebTemp (Chebyshev graph conv, temporal weights).

Math: out[b,v,o] = sum_{k,t,f} T_k(L)x0[:,t,f,b] w[f,k,t,o] + bias[o]
with x0 = inputs permuted to [V, T*Fin*B] and T_k the Chebyshev recurrence.

Clenshaw reformulation (weights contracted first):
  z_k[v,b,o] = sum_{t,f} x0[v,t,f,b] w[f,k,t,o]
  b3 = z3; b2 = z2 + 2 L b3; b1 = z1 + 2 L b2 - b3; out = z0 + L b1 - b2 + bias

Sharding: 8 cores = 4 pairs. Pair p owns batches [4p, 4p+4); within the pair
the graph rows are split in half (core 2p: rows [0, V/2), core 2p+1 the rest).
The Clenshaw iterates b3/b2/b1 live in pair-SHARED HBM tensors
(addr_space="Shared": cores (2k, 2k+1) see one physical buffer), so each
core writes only its half and gathers from the full tensor. Cross-core
ordering is a tiny per-pair AllGather barrier before each phase's gathers.

Everything on the SpMM path is bf16: gather rows are 4 batches x 64 Fout x 2B
= 512B (full DMA descriptor efficiency) and all matmuls run at 1 cycle/row.
"""
import sys

sys.path.insert(0, "/opt/trn_rl_repo")

from contextlib import ExitStack  # noqa: E402

import ml_dtypes  # noqa: E402
import numpy as np  # noqa: E402

from concourse import bacc, bass, mybir, tile  # noqa: E402
from concourse.bass_utils import run_bass_kernel_spmd  # noqa: E402

P = 128
N_CORES = 8
FP32 = mybir.dt.float32
BF16 = mybir.dt.bfloat16
I32 = mybir.dt.int32
I16 = mybir.dt.int16

# Problem dims (hardcoded per spec)
B, V, T, FIN = 16, 12288, 4, 64
KV, KT, FOUT = 4, 4, 64
VH = V // 2                # rows per core
NT = VH // P               # out-tiles per core (48)
BG = 4                     # batches per pair
F = BG * FOUT              # spmm row width (256 bf16 = 512B)
C = T * FIN                # z contraction dim (256)
PAIR_GROUPS = [[0, 1], [2, 3], [4, 5], [6, 7]]
CHUNKS_PER_PIECE = 8       # 1024 gather indices per instruction
DMA_SCRATCH = 16384        # SWDGE ring: 1024 descriptors
WGRP = 8                   # out-tiles per batched shared-HBM write


def _preprocess_lap(lap_rows, lap_cols, lap_vals):
    """Split nnz by row-half, sort by local row, pad to a common per-tile
    chunk structure (identical across cores so one SPMD program serves all).

    Returns (per_parity list of (gidx_wrapped, growl, gval), chunks_per_tile).
    """
    halves = []
    counts_h = []
    for h in (0, 1):
        lo, hi = h * VH, (h + 1) * VH
        m = (lap_rows >= lo) & (lap_rows < hi)
        lrows = lap_rows[m] - lo
        order = np.argsort(lrows, kind="stable")
        lrows = lrows[order]
        cols = lap_cols[m][order]
        vals = lap_vals[m][order]
        counts = np.bincount(lrows // P, minlength=NT)
        halves.append((lrows, cols, vals, counts))
        counts_h.append(counts)
    chunks_per_tile = [
        max(1, int(-(-counts_h[0][t] // P)), int(-(-counts_h[1][t] // P)))
        for t in range(NT)
    ]
    nchunk = sum(chunks_per_tile)
    nnzp = nchunk * P
    out = []
    for lrows, cols, vals, counts in halves:
        gidx = np.zeros(nnzp, np.int16)
        growl = np.zeros(nnzp, np.float32)
        gval = np.zeros(nnzp, np.float32)
        starts = np.zeros(NT + 1, np.int64)
        np.cumsum(counts, out=starts[1:])
        pos = 0
        for t in range(NT):
            n = int(counts[t])
            s = int(starts[t])
            gidx[pos:pos + n] = cols[s:s + n]
            growl[pos:pos + n] = (lrows[s:s + n] - t * P).astype(np.float32)
            gval[pos:pos + n] = vals[s:s + n]
            pos += chunks_per_tile[t] * P
        assert pos == nnzp
        gidx_w = np.tile(gidx.reshape(-1, 16).T.copy(), (8, 1))  # [128, nnzp/16]
        growl_m = growl.reshape(nchunk, P).T.copy()
        gval_m = gval.reshape(nchunk, P).T.copy()
        out.append((np.ascontiguousarray(gidx_w),
                    np.ascontiguousarray(growl_m),
                    np.ascontiguousarray(gval_m)))
    return out, chunks_per_tile


def build_program(chunks_per_tile, has_bias, n_cores=N_CORES):
    nt = NT
    nchunk = sum(chunks_per_tile)
    nnzp = nchunk * P
    nc = bacc.Bacc("TRN2", target_bir_lowering=False, debug=False,
                   num_devices=n_cores, dynamic_dma_scratch_size=DMA_SCRATCH)

    xt_d = nc.dram_tensor("xt", [BG, 2, P, VH], BF16, kind="ExternalInput")
    wz_d = nc.dram_tensor("wz", [P, 2, KV * FOUT], BF16, kind="ExternalInput")
    onesb_d = nc.dram_tensor("onesb", [1, P], BF16, kind="ExternalInput")
    biasw_d = nc.dram_tensor("biasw", [1, KV * FOUT], BF16, kind="ExternalInput")
    iota_d = nc.dram_tensor("iota128", [P, P], BF16, kind="ExternalInput")
    offt_d = nc.dram_tensor("offt", [1, 1], I32, kind="ExternalInput")
    gidx_d = nc.dram_tensor("gidx", [P, nnzp // 16], I16, kind="ExternalInput")
    growl_d = nc.dram_tensor("growl", [P, nchunk], FP32, kind="ExternalInput")
    gval1_d = nc.dram_tensor("gval1", [P, nchunk], FP32, kind="ExternalInput")
    gval2_d = nc.dram_tensor("gval2", [P, nchunk], FP32, kind="ExternalInput")
    out_d = nc.dram_tensor("out", [VH, F], FP32, kind="ExternalOutput")

    # pair-shared Clenshaw iterates (both cores of a pair see one buffer)
    bsh = [nc.dram_tensor(f"bsh{k}", [V, F], BF16, kind="Internal",
                          addr_space="Shared") for k in range(3)]
    bin_d = [nc.dram_tensor(f"bin{k}", [1, 16], BF16, kind="Internal")
             for k in range(3)]
    bout_d = [nc.dram_tensor(f"bout{k}", [2, 16], BF16, kind="Internal")
              for k in range(3)]

    with tile.TileContext(nc) as tc, ExitStack() as ctx:
        const = ctx.enter_context(tc.tile_pool(name="const", bufs=1))
        zres = ctx.enter_context(tc.tile_pool(name="zres", bufs=1))
        xpool = ctx.enter_context(tc.tile_pool(name="x", bufs=2))
        gpool = ctx.enter_context(tc.tile_pool(name="gbuf", bufs=4))
        spool = ctx.enter_context(tc.tile_pool(name="sel", bufs=3))
        opool = ctx.enter_context(tc.tile_pool(name="ostg", bufs=2))
        bpool = ctx.enter_context(tc.tile_pool(name="bounce", bufs=1))
        psz = ctx.enter_context(tc.tile_pool(name="psz", bufs=3, space="PSUM"))
        pss = ctx.enter_context(tc.tile_pool(name="pss", bufs=4, space="PSUM"))

        # constants + metadata resident in SBUF
        iota_sb = const.tile([P, P], BF16, tag="iota")
        nc.sync.dma_start(iota_sb[:], iota_d[:, :])
        ones_sb = const.tile([1, P], BF16, tag="ones")
        nc.sync.dma_start(ones_sb[:], onesb_d[:, :])
        biasw_sb = const.tile([1, KV * FOUT], BF16, tag="biasw")
        nc.sync.dma_start(biasw_sb[:], biasw_d[:, :])
        wz_sb = const.tile([P, 2, KV * FOUT], BF16, tag="wz")
        nc.sync.dma_start(wz_sb[:], wz_d[:, :, :])
        gidx_sb = const.tile([P, nnzp // 16], I16, tag="gidx")
        nc.sync.dma_start(gidx_sb[:], gidx_d[:, :])
        growl_sb = const.tile([P, nchunk], FP32, tag="growl")
        nc.sync.dma_start(growl_sb[:], growl_d[:, :])
        gval1_sb = const.tile([P, nchunk], FP32, tag="gval1")
        nc.sync.dma_start(gval1_sb[:], gval1_d[:, :])
        gval2_sb = const.tile([P, nchunk], FP32, tag="gval2")
        nc.sync.dma_start(gval2_sb[:], gval2_d[:, :])

        # my row offset into the shared [V, F] tensors (0 or VH); loaded on
        # the Pool engine, which issues the symbolic shared writes (SWDGE)
        off_reg = nc.gpsimd.alloc_register("slab_off")
        nc.gpsimd.reg_load(off_reg, offt_d[0:1, 0:1])
        off = nc.gpsimd.snap(off_reg, donate=True, min_val=0, max_val=VH)

        # all z_k resident in SBUF: [P, nt, KV, BG, FOUT] bf16 (96KB/partition)
        z_sb = zres.tile([P, nt, KV, BG, FOUT], BF16, tag="z")

        # ---------- phase Z: z_k = x0 @ w_k (+ bias folded into z0) ----------
        VHH = VH // 2
        for b in range(BG):
          for half in range(2):
            v0 = half * VHH
            xb = xpool.tile([P, 2, VHH], BF16, tag="xb")
            nc.sync.dma_start(
                xb[:], xt_d[b, :, :, v0:v0 + VHH].rearrange("c p v -> p c v"))
            for vt in range(half * nt // 2, (half + 1) * nt // 2):
                zps = psz.tile([P, KV * FOUT], FP32, tag="zps")
                for cc in range(2):
                    nc.tensor.matmul(zps[:],
                                     lhsT=xb[:, cc, vt * P - v0:(vt + 1) * P - v0],
                                     rhs=wz_sb[:, cc, :],
                                     start=(cc == 0),
                                     stop=(cc == 1 and not has_bias))
                if has_bias:
                    nc.tensor.matmul(zps[:], lhsT=ones_sb[:, :],
                                     rhs=biasw_sb[:, :], start=False, stop=True)
                # PSUM->SBUF cast copies alternate DVE / Activation
                if vt % 2 == 0:
                    nc.vector.tensor_copy(
                        z_sb[:, vt, :, b, :],
                        zps[:].rearrange("p (k o) -> p k o", o=FOUT))
                else:
                    nc.scalar.activation(
                        out=z_sb[:, vt, :, b, :],
                        in_=zps[:].rearrange("p (k o) -> p k o", o=FOUT),
                        func=mybir.ActivationFunctionType.Copy)

        shared_writes = {0: [], 1: [], 2: []}

        def write_half(kidx, kslot, grp):
            """Batched write of WGRP tiles of z-slot kslot to shared bsh[kidx]."""
            g0 = grp * WGRP
            dst = bsh[kidx][bass.ds(off + g0 * P, WGRP * P), :] \
                .rearrange("(t p) f -> p t f", p=P)
            src = z_sb[:, g0:g0 + WGRP, kslot, :, :] \
                .rearrange("p t b o -> p t (b o)")
            w = nc.gpsimd.dma_start(dst, src)
            shared_writes[kidx].append(w)

        for grp in range(nt // WGRP):
            write_half(0, 3, grp)

        def pair_barrier(k):
            bsb = bpool.tile([1, 16], BF16, tag=f"bsb{k}")
            rd = nc.sync.dma_start(bsb[:], bsh[k][0:1, 0:16])
            # the bounce read must follow ALL my writes to bsh[k], not just
            # the group that happens to overlap row 0
            for w in shared_writes[k]:
                bass._add_dep_helper(rd.ins, w.ins, sync=True,
                                     reason="barrier after all shared writes")
            nc.sync.dma_start(bin_d[k][0:1, :], bsb[:])
            return nc.gpsimd.collective_compute(
                "AllGather", mybir.AluOpType.bypass, PAIR_GROUPS,
                ins=[bin_d[k][0:1, :]], outs=[bout_d[k][:, :]])

        # ---------- spmm phases ----------
        def spmm_phase(src_d, vals_sb, cc_inst, combine):
            state = {"gb": None, "base": 0, "len": 0}

            def ensure_piece(c):
                while state["gb"] is None or c >= state["base"] + state["len"]:
                    base = 0 if state["gb"] is None else state["base"] + state["len"]
                    plen = min(CHUNKS_PER_PIECE, nchunk - base)
                    gb = gpool.tile([P, plen, F], BF16, tag="gb")
                    s0 = base * P
                    nidx = plen * P
                    g = nc.gpsimd.dma_gather(
                        out_ap=gb[:],
                        in_ap=src_d[:, :],
                        idxs_ap=gidx_sb[:, s0 // 16:(s0 + nidx) // 16],
                        num_idxs=nidx,
                        num_idxs_reg=nidx,
                        elem_size=F,
                    )
                    bass._add_dep_helper(g.ins, cc_inst.ins, sync=True,
                                         reason="pair barrier before gather")
                    state.update(gb=gb, base=base, len=plen)
                return state["gb"], state["base"]

            ci = 0
            for tt in range(nt):
                nck = chunks_per_tile[tt]
                ps = pss.tile([P, F], FP32, tag="ps")
                for k in range(nck):
                    col = ci + k
                    gb, base = ensure_piece(col)
                    sT = spool.tile([P, P], BF16, tag="sT")
                    nc.vector.tensor_scalar(
                        out=sT[:], in0=iota_sb[:],
                        scalar1=growl_sb[:, col:col + 1],
                        scalar2=vals_sb[:, col:col + 1],
                        op0=mybir.AluOpType.is_equal,
                        op1=mybir.AluOpType.mult,
                    )
                    nc.tensor.matmul(ps[:], lhsT=sT[:], rhs=gb[:, col - base, :],
                                     start=(k == 0), stop=(k == nck - 1))
                combine(tt, ps)
                ci += nck

        def zslot(vt, k):
            return z_sb[:, vt, k, :, :].rearrange("p b o -> p (b o)")

        def ps3(ps):
            return ps[:].rearrange("p (b o) -> p b o", o=FOUT)

        # phase 1: b2 = z2 + 2 L b3   (result overwrites z2 slot)
        cc0 = pair_barrier(0)

        def combine1(tt, ps):
            nc.vector.tensor_tensor(out=zslot(tt, 2), in0=zslot(tt, 2),
                                    in1=ps[:], op=mybir.AluOpType.add)
            if (tt + 1) % WGRP == 0:
                write_half(1, 2, tt // WGRP)

        spmm_phase(bsh[0], gval2_sb, cc0, combine1)

        # phase 2: b1 = z1 + 2 L b2 - b3   (result overwrites z1 slot)
        cc1 = pair_barrier(1)

        def combine2(tt, ps):
            nc.vector.tensor_tensor(out=zslot(tt, 1), in0=zslot(tt, 1),
                                    in1=ps[:], op=mybir.AluOpType.add)
            nc.vector.tensor_tensor(out=zslot(tt, 1), in0=zslot(tt, 1),
                                    in1=zslot(tt, 3), op=mybir.AluOpType.subtract)
            if (tt + 1) % WGRP == 0:
                write_half(2, 1, tt // WGRP)

        spmm_phase(bsh[1], gval2_sb, cc1, combine2)

        # phase 3: out = z0 + L b1 - b2 + bias
        cc2 = pair_barrier(2)

        def combine3(tt, ps):
            ot = opool.tile([P, F], FP32, tag="ot")
            nc.vector.tensor_tensor(out=ot[:], in0=ps[:], in1=zslot(tt, 2),
                                    op=mybir.AluOpType.subtract)
            nc.vector.tensor_tensor(out=ot[:], in0=ot[:], in1=zslot(tt, 0),
                                    op=mybir.AluOpType.add)
            nc.sync.dma_start(out_d[tt * P:(tt + 1) * P, :], ot[:])

        spmm_phase(bsh[2], gval1_sb, cc2, combine3)

    nc.compile()
    return nc


def make_host_inputs(inputs, weight, bias, lap_vals, lap_rows, lap_cols):
    per_parity, chunks = _preprocess_lap(
        np.asarray(lap_rows), np.asarray(lap_cols),
        np.asarray(lap_vals, np.float32))
    w = np.asarray(weight, np.float32)
    # wz[(t,f) split cc, (k,o)]
    wz = np.transpose(w, (2, 0, 1, 3)).reshape(C, KV * FOUT)
    wz = np.ascontiguousarray(
        wz.reshape(2, P, KV * FOUT).transpose(1, 0, 2)).astype(ml_dtypes.bfloat16)
    biasw = np.zeros((1, KV * FOUT), np.float32)
    biasw[0, :FOUT] = np.asarray(bias, np.float32)
    biasw = biasw.astype(ml_dtypes.bfloat16)
    onesb = np.ones((1, P), ml_dtypes.bfloat16)
    iota128 = np.ascontiguousarray(
        np.broadcast_to(np.arange(P, dtype=np.float32)[None, :],
                        (P, P))).astype(ml_dtypes.bfloat16)
    x = np.asarray(inputs, np.float32)
    in_maps = []
    for r in range(N_CORES):
        pair, h = r // 2, r % 2
        gidx_w, growl_m, gval_m = per_parity[h]
        # xt[b, cc, cl, v] = x[4p+b, h*VH + v, t, f], c=(t,f)=cc*128+cl
        xs = x[BG * pair:BG * (pair + 1), h * VH:(h + 1) * VH]  # [4, VH, T, FIN]
        xt = xs.reshape(BG, VH, C).transpose(0, 2, 1).reshape(BG, 2, P, VH)
        m = {
            "xt": np.ascontiguousarray(xt).astype(ml_dtypes.bfloat16),
            "wz": wz,
            "biasw": biasw,
            "onesb": onesb,
            "iota128": iota128,
            "offt": np.array([[h * VH]], np.int32),
            "gidx": gidx_w,
            "growl": growl_m,
            "gval1": gval_m,
            "gval2": np.ascontiguousarray(2.0 * gval_m),
        }
        in_maps.append(m)
    return in_maps, chunks


_CACHE = {}


def _get_program(chunks, has_bias):
    key = (tuple(chunks), has_bias)
    if key not in _CACHE:
        _CACHE[key] = build_program(list(chunks), has_bias)
    return _CACHE[key]


def kernel(inputs, weight, bias, lap_vals, lap_rows, lap_cols):
    in_maps, chunks = make_host_inputs(inputs, weight, bias, lap_vals,
                                       lap_rows, lap_cols)
    nc = _get_program(chunks, bool(np.any(np.asarray(bias))))
    res = run_bass_kernel_spmd(nc, in_maps, list(range(N_CORES)))
    out = np.empty((B, V, FOUT), np.float32)
    for r in range(N_CORES):
        pair, h = r // 2, r % 2
        o = res.results[r]["out"].reshape(VH, BG, FOUT)
        out[BG * pair:BG * (pair + 1), h * VH:(h + 1) * VH, :] = \
            o.transpose(1, 0, 2)
    return np.ascontiguousarray(out)


def time_kernel(inputs_dict, iters=3):
    """Wall-clock repeated executions of the cached program (ns per run)."""
    import time

    in_maps, chunks = make_host_inputs(**inputs_dict)
    nc = _get_program(chunks, bool(np.any(np.asarray(inputs_dict["bias"]))))
    times = []
    for _ in range(iters):
        t0 = time.perf_counter()
        run_bass_kernel_spmd(nc, in_maps, list(range(N_CORES)))
        times.append(time.perf_counter() - t0)
    return min(times) * 1e9
